# revision 1
# baseline (speedup 1.0000x reference)
"""Trainium2 Bass kernel for nn_DifferentiableKalmanFilter.

Strategy
--------
The 4x4 covariance recursion is batch-independent and, by x/y symmetry,
collapses to two scalar gain sequences k_p(t), k_v(t) computed on the host.
Per batch row the filter is a 2-state linear recurrence per coordinate:

    s_t = s_{t-1} @ M_t + z_t * g_t,   s = [p, v],
    M_t = [[1, 0], [dt - k_p(t), 1 - k_v(t)]],  g_t = [k_p(t), k_v(t)]

Unrolling over a chunk of L=124 timesteps turns the chunk into one matmul
out[b, :] = [z_window | carry] @ W with host-precomputed W (per
coordinate; x/y share W). The carried state (p, v) rides along as four
fp16 contraction rows (hi/lo pairs), making K exactly 128 for full
chunks. A small second matmul per chunk computes the next carry directly
in its transposed (2, batch) layout; chunk 0's carry (p0, v=0) is baked
into the host-built input.

Precision: all matmul operands are fp16 *pairs* (hi + lo, ~22 mantissa
bits), pre-scaled by 2^8 to stay clear of fp16 subnormals; products
accumulate in fp32 PSUM and are rescaled by 2^-16 during the PSUM->SBUF
copy. Three products (Xh@Wh + Xh@Wl + Xl@Wh) give ~1e-6 relative error
at full bf16-class matmul throughput.

The contraction needs z time-major, so the host passes pre-transposed,
pre-split, pre-stacked copies of pred_vel (zh/zl) - every device DMA is
contiguous and no on-device transposes are needed.

Sharding: pure data parallel over batch across 8 cores (1024 rows/core).
The device writes one (BC, 4, T) output tensor (pos_x, vel_x, pos_y,
vel_y planes); the host interleaves coordinates at the end.
"""

import numpy as np

import concourse.bass as bass
import concourse.tile as tile
from concourse import bacc, mybir
from concourse.bass_utils import run_bass_kernel_spmd

# Problem shape (hardcoded per harness contract)
B = 8192
T = 1024
NCORES = 8
BC = B // NCORES  # 1024 batch rows per core
L = 124
CH = [(c * L, L) for c in range(T // L)] + (
    [(T - T % L, T % L)] if T % L else []
)  # [(t0, Lc)] chunks; 8 x 124 + 32
NCH = len(CH)


def _make_groups(bb):
    """Output-store chunk groups for batch-block bb, staggered by bb so the
    ~1MB store DMAs spread evenly over the kernel instead of bursting."""
    bnds = [0]
    for c in range(1, NCH):
        if CH[c][1] != L or (c - (bb % 4)) % 4 == 0:
            bnds.append(c)
    bnds.append(NCH)
    return [(s, e) for s, e in zip(bnds, bnds[1:]) if s < e]


GMAXLEN = max(
    sum(CH[c][1] for c in range(s, e))
    for bb in range(4)
    for s, e in _make_groups(bb)
)
SZ = 256.0  # input scale (keeps fp16 operands away from subnormals)
SW = 256.0  # weight scale
INV = float(2.0**-16)  # psum rescale = 1/(SZ*SW)


# ---------------------------------------------------------------- host math
def _gains(dt, q_pos, q_vel, r_vel):
    """Scalar Kalman gain sequences in float64 (exact vs fp32 reference)."""
    dt = float(dt)
    r_reg = float(np.float32(r_vel) + np.float32(1e-6))
    q_pos = float(q_pos)
    q_vel = float(q_vel)
    a, b, c = 1.0, 0.0, 1.0  # P blocks [[a, b], [b, c]] per coordinate
    kp = np.zeros(T)
    kv = np.zeros(T)
    for t in range(T):
        ap = a + 2 * dt * b + dt * dt * c + q_pos
        bp = b + dt * c
        cp = c + q_vel
        den = cp + r_reg
        kp[t] = bp / den
        kv[t] = cp / den
        a = ap - kp[t] * bp
        b = bp * r_reg / den
        c = cp * r_reg / den
    return kp, kv


def _split_f16(x, scale):
    """Split scale*x into two float16 terms (hi, lo)."""
    x = (np.asarray(x, dtype=np.float64) * scale).astype(np.float32)
    h = x.astype(np.float16)
    lo = (x - h.astype(np.float32)).astype(np.float16)
    return h, lo


def _build_weights(kp, kv, dt):
    """Per-chunk weight matrices.

    W64 rows: [z(0:Lc) | p, v, p, v] - the carry rows pair with stack rows
    [ph, vh, pl, vl]; identical weights apply to the hi and lo fp16 halves.
    Cols per coordinate: [pos(0:Lc) | vel(Lc:2Lc)]. The p-carry weight is
    exactly 1 on pos cols, 0 on vel cols.

    Returns (w_full, w_last, wc_full, wc_last, chunk_map) where wc are the
    carry-update weights = W columns [pos(Lc-1), vel(2Lc-1)].
    """
    dt = float(dt)
    w64 = []
    for t0, Lc in CH:
        U = np.zeros((Lc, 2))
        C = np.eye(2)
        Wz = np.zeros((Lc, Lc, 2))
        Wp = np.zeros((Lc, 2))
        Wv = np.zeros((Lc, 2))
        for u in range(Lc):
            t = t0 + u
            M = np.array([[1.0, 0.0], [dt - kp[t], 1.0 - kv[t]]])
            U[:u] = U[:u] @ M
            U[u] = (kp[t], kv[t])
            C = C @ M
            Wz[: u + 1, u, :] = U[: u + 1]
            Wp[u, :] = C[0]  # = [1, 0] always
            Wv[u, :] = C[1]
        W = np.zeros((Lc + 4, 2 * Lc))
        W[:Lc, :Lc] = Wz[:, :, 0]
        W[:Lc, Lc:] = Wz[:, :, 1]
        for r, Wc in ((0, Wp), (1, Wv), (2, Wp), (3, Wv)):
            W[Lc + r, :Lc] = Wc[:, 0]
            W[Lc + r, Lc:] = Wc[:, 1]
        w64.append(W)

    # dedupe the full-size chunks (gains converge -> steady chunks share W)
    chunk_map = []
    uniq = []
    nfull = sum(1 for _, Lc in CH if Lc == L)
    for c in range(nfull):
        found = None
        for ui, u in enumerate(uniq):
            if np.array_equal(w64[c], w64[u]):
                found = ui
                break
        if found is None:
            uniq.append(c)
            found = len(uniq) - 1
        chunk_map.append(found)

    def pack(Wm, Lc):
        out = np.zeros((2, Lc + 4, 2 * Lc), dtype=np.float16)
        out[0], out[1] = _split_f16(Wm, SW)
        return out

    def packc(Wm, Lc):
        # carry-update weights at scale 1: cps = (z*2^8) @ Wc = carry*2^8,
        # directly at the z-scale the next chunk's carry rows need
        out = np.zeros((2, Lc + 4, 2), dtype=np.float16)
        out[0], out[1] = _split_f16(Wm[:, [Lc - 1, 2 * Lc - 1]], 1.0)
        return out

    w_full = np.stack([pack(w64[u], L) for u in uniq])
    wc_full = np.stack([packc(w64[u], L) for u in uniq])
    if len(CH) > nfull:
        _, Ll = CH[-1]
        w_last = pack(w64[-1], Ll)
        wc_last = packc(w64[-1], Ll)
    else:
        w_last = wc_last = None
    return w_full, w_last, wc_full, wc_last, chunk_map


# ---------------------------------------------------------------- bass build
def _build_nc(nuf, chunk_map, has_last):
    f32 = mybir.dt.float32
    f16 = mybir.dt.float16
    AO = mybir.AluOpType

    nc = bacc.Bacc(
        "TRN2",
        target_bir_lowering=False,
        debug=False,
        enable_asserts=False,
    )
    zh_d = nc.dram_tensor("zh", [2, NCH, 128, BC], f16, kind="ExternalInput").ap()
    zl_d = nc.dram_tensor("zl", [2, NCH, 128, BC], f16, kind="ExternalInput").ap()
    w_d = nc.dram_tensor("w", [nuf, 2, L + 4, 2 * L], f16, kind="ExternalInput").ap()
    wc_d = nc.dram_tensor("wc", [nuf, 2, L + 4, 2], f16, kind="ExternalInput").ap()
    if has_last:
        _, Ll = CH[-1]
        wl_d = nc.dram_tensor(
            "wlast", [2, Ll + 4, 2 * Ll], f16, kind="ExternalInput"
        ).ap()
        wcl_d = nc.dram_tensor(
            "wclast", [2, Ll + 4, 2], f16, kind="ExternalInput"
        ).ap()
    out_all = nc.dram_tensor("out", [BC, 4, T], f32, kind="ExternalOutput").ap()

    with tile.TileContext(nc) as tc:
        with (
            tc.tile_pool(name="wpool", bufs=1) as wpool,
            tc.tile_pool(name="stacks", bufs=1) as spool,
            tc.tile_pool(name="splits", bufs=8) as xpool,
            tc.tile_pool(name="outp", bufs=10) as opool,
            tc.tile_pool(name="mpsum", bufs=4, space="PSUM") as mpsum_pool,
            tc.tile_pool(name="cpsum", bufs=2, space="PSUM") as cpsum_pool,
        ):
            w_t = wpool.tile([L + 4, nuf, 2, 2 * L], f16)
            nc.gpsimd.dma_start(w_t[:], w_d.rearrange("u s k n -> k u s n"))
            wc_t = wpool.tile([L + 4, nuf, 2, 2], f16)
            nc.gpsimd.dma_start(wc_t[:], wc_d.rearrange("u s k n -> k u s n"))
            if has_last:
                _, Ll = CH[-1]
                wl_t = wpool.tile([Ll + 4, 2, 2 * Ll], f16)
                nc.gpsimd.dma_start(wl_t[:], wl_d.rearrange("s k n -> k s n"))
                wcl_t = wpool.tile([Ll + 4, 2, 2], f16)
                nc.gpsimd.dma_start(wcl_t[:], wcl_d.rearrange("s k n -> k s n"))

            # z stacks: rows [z(0:Lc) | ph vh pl vl | pad], pre-stacked and
            # zero-padded by the host (chunk 0 carry rows hold p0); one tile
            # per (chunk, coord, h/l), DMA'd in two chunks ahead of use so
            # prefetch does not starve the current chunk's bandwidth
            stacks = {}
            _eng = [nc.gpsimd, nc.gpsimd, nc.gpsimd]

            def issue_stacks(c):
                Kc = CH[c][1] + 4
                for cd in range(2):
                    for hl in range(2):
                        stk = spool.tile(
                            [128, BC], f16, tag=f"stk_{c}_{cd}_{hl}"
                        )
                        zsrc = zh_d if hl == 0 else zl_d
                        if c < 2:
                            # first chunks via HWDGE (idle at startup; the
                            # gpsimd SWDGE queue serializes ~1us/DMA descgen)
                            eng = nc.sync if (cd * 2 + hl) % 2 else nc.scalar
                        else:
                            eng = nc.gpsimd
                        eng.dma_start(stk[0:Kc, :], zsrc[cd, c, 0:Kc, :])
                        stacks[(c, cd, hl)] = stk

            for _c in range(NCH):
                issue_stacks(_c)

            group_out = [None] * (BC // 128)
            ndma = 0  # round-robin HWDGE issuer for out DMAs

            def hwdge():
                nonlocal ndma
                ndma += 1
                return nc.sync if ndma % 2 else nc.scalar

            def _wslices(c):
                full = CH[c][1] == L
                ci = chunk_map[c] if full else None

                def wmain(hl, ci=ci, full=full):
                    return w_t[:, ci, hl] if full else wl_t[:, hl]

                def wcarry(hl, ci=ci, full=full):
                    return wc_t[:, ci, hl] if full else wcl_t[:, hl]

                return wmain, wcarry

            def process_carry(c, g):
                """Carry chain for chunk c -> c+1, one 512-row batch group."""
                t0, Lc = CH[c]
                K = Lc + 4
                wmain, wcarry = _wslices(c)
                if c + 1 < NCH:
                    Lc1 = CH[c + 1][1]
                    for cd in range(2):
                        nxt = stacks[(c + 1, cd, 0)]
                        if True:
                            gsl = slice(g * 512, (g + 1) * 512)
                            cps = cpsum_pool.tile([2, 512], f32, tag="cps")
                            nc.tensor.matmul(
                                cps[:], wcarry(0), stacks[(c, cd, 0)][0:K, gsl],
                                start=True, stop=False,
                            )
                            nc.tensor.matmul(
                                cps[:], wcarry(1), stacks[(c, cd, 0)][0:K, gsl],
                                start=False, stop=False,
                            )
                            nc.tensor.matmul(
                                cps[:], wcarry(0), stacks[(c, cd, 1)][0:K, gsl],
                                start=False, stop=True,
                            )
                            # split carry*2^8 into fp16 [ph,vh] + [pl,vl]
                            # rows (partition-0 scratch, then DMA into the
                            # next stack: engine writes can't start at
                            # partition Lc1)
                            h_t = xpool.tile([2, 512], f16, tag="h")
                            l_t = xpool.tile([2, 512], f16, tag="l")
                            nc.scalar.mul(h_t[:], cps[:], 1.0)
                            nc.vector.tensor_tensor(
                                l_t[:], cps[:], h_t[:], AO.subtract
                            )
                            hwdge().dma_start(nxt[Lc1 : Lc1 + 2, gsl], h_t[:])
                            hwdge().dma_start(
                                nxt[Lc1 + 2 : Lc1 + 4, gsl], l_t[:]
                            )

            def process_mains(c, g):
                # main chunk outputs (4 output planes x group-t window);
                # out tiles span a group of chunks so the store DMA moves
                # ~1 MB with ~2 KB contiguous runs (groups staggered by bb)
                t0, Lc = CH[c]
                K = Lc + 4
                wmain, wcarry = _wslices(c)
                for bb in range(g * 4, g * 4 + 4):
                    grp = next(
                        (s, e) for s, e in _make_groups(bb) if s <= c < e
                    )
                    gi_t = CH[c][0] - CH[grp[0]][0]  # t offset within group
                    bsl = slice(bb * 128, (bb + 1) * 128)
                    mps = mpsum_pool.tile([128, 4 * Lc], f32, tag="mps")
                    for cd in range(2):
                        osl = slice(cd * 2 * Lc, (cd + 1) * 2 * Lc)
                        nc.tensor.matmul(
                            mps[:, osl], stacks[(c, cd, 0)][0:K, bsl],
                            wmain(0), start=True, stop=False,
                        )
                        nc.tensor.matmul(
                            mps[:, osl], stacks[(c, cd, 0)][0:K, bsl],
                            wmain(1), start=False, stop=False,
                        )
                        nc.tensor.matmul(
                            mps[:, osl], stacks[(c, cd, 1)][0:K, bsl],
                            wmain(0), start=False, stop=True,
                        )
                    if c == grp[0]:
                        group_out[bb] = opool.tile(
                            [128, 4, GMAXLEN], f32, tag="out", name=f"out_{c}_{bb}"
                        )
                    out_t = group_out[bb]
                    # uniform rescale on the way out of PSUM (DVE/ACT split)
                    dst = out_t[:, :, gi_t : gi_t + Lc]
                    srcv = mps[:].rearrange("b (o t) -> b o t", o=4)
                    if bb % 3 == 0:
                        nc.vector.tensor_scalar_mul(dst, srcv, INV)
                    else:
                        nc.scalar.mul(dst, srcv, INV)
                    if c == grp[1] - 1:
                        tg0 = CH[grp[0]][0]
                        twid = t0 + Lc - tg0
                        hwdge().dma_start(
                            out_all[
                                bb * 128 : (bb + 1) * 128, :, tg0 : tg0 + twid
                            ],
                            out_t[:, :, 0:twid],
                        )

            # batch-group 1 runs SKEW chunks behind group 0 so the final
            # stores of one half drain while the other half still computes
            SKEW = 1
            for step in range(NCH + SKEW):
                for g in range(2):
                    c = step - g * SKEW
                    if 0 <= c < NCH:
                        process_carry(c, g)
                        process_mains(c, g)
    nc.compile()
    return nc


# ---------------------------------------------------------------- entry
def _prepare(pred_vel, dt, p0, q_pos, q_vel, r_vel):
    kp, kv = _gains(dt, q_pos, q_vel, r_vel)
    w_full, w_last, wc_full, wc_last, chunk_map = _build_weights(kp, kv, dt)
    nuf = w_full.shape[0]

    pred_vel = np.asarray(pred_vel, dtype=np.float32)
    p0 = np.asarray(p0, dtype=np.float32)
    in_maps = []
    for i in range(NCORES):
        pv = pred_vel[i * BC : (i + 1) * BC]  # (BC, T, 2)
        zt = np.ascontiguousarray(pv.transpose(2, 1, 0)) * np.float32(SZ)
        zth = zt.astype(np.float16)
        ztl = (zt - zth.astype(np.float32)).astype(np.float16)
        zh = np.zeros((2, NCH, 128, BC), dtype=np.float16)
        zl = np.zeros((2, NCH, 128, BC), dtype=np.float16)
        for c, (t0, Lc) in enumerate(CH):
            zh[:, c, 0:Lc, :] = zth[:, t0 : t0 + Lc, :]
            zl[:, c, 0:Lc, :] = ztl[:, t0 : t0 + Lc, :]
        # bake the initial carry (p = p0, v = 0) into chunk 0's rows
        p0h, p0l = _split_f16(p0[i * BC : (i + 1) * BC].T, SZ)  # (2, BC)
        Lc0 = CH[0][1]
        zh[:, 0, Lc0 + 0, :] = p0h
        zh[:, 0, Lc0 + 2, :] = p0l
        m = {"zh": zh, "zl": zl, "w": w_full, "wc": wc_full}
        if w_last is not None:
            m["wlast"] = w_last
            m["wclast"] = wc_last
        in_maps.append(m)
    return nuf, chunk_map, w_last is not None, in_maps


def run(pred_vel, dt, p0, q_pos, q_vel, r_vel, trace=False, **spmd_kwargs):
    nuf, chunk_map, has_last, in_maps = _prepare(
        pred_vel, dt, p0, q_pos, q_vel, r_vel
    )
    nc = _build_nc(nuf, chunk_map, has_last)
    res = run_bass_kernel_spmd(
        nc, in_maps, core_ids=list(range(NCORES)), trace=trace, **spmd_kwargs
    )
    pos = np.empty((B, T, 2), dtype=np.float32)
    vel = np.empty((B, T, 2), dtype=np.float32)
    for i in range(NCORES):
        o = res.results[i]["out"]  # (BC, 4, T)
        sl = slice(i * BC, (i + 1) * BC)
        pos[sl, :, 0] = o[:, 0]
        vel[sl, :, 0] = o[:, 1]
        pos[sl, :, 1] = o[:, 2]
        vel[sl, :, 1] = o[:, 3]
    return (pos, vel), res


def kernel(pred_vel, dt, p0, q_pos, q_vel, r_vel):
    (pos, vel), _ = run(pred_vel, dt, p0, q_pos, q_vel, r_vel, trace=False)
    return pos, vel



# revision 2
# speedup vs baseline: 1.0491x; 1.0491x over previous
"""Trainium2 Bass kernel for nn_DifferentiableKalmanFilter (v2).

Strategy
--------
The 4x4 covariance recursion is batch-independent and, by x/y symmetry,
collapses to two scalar gain sequences k_p(t), k_v(t) computed on the host.
Per batch row the filter is a 2-state linear recurrence per coordinate:

    s_t = s_{t-1} @ M_t + z_t * g_t,   s = [p, v],
    M_t = [[1, 0], [dt - k_p(t), 1 - k_v(t)]],  g_t = [k_p(t), k_v(t)]

Unrolled over a chunk of L=126 timesteps this is a matmul with
host-precomputed weights. The tolerance (2e-2) allows a single fp16
product (no hi/lo splitting, no scaling): z, W, the carried state and the
output are all plain fp16; PSUM accumulates in f32.

Layout: weights are the stationary operand [K=2+L, M=2+L], the z/carry
stack [K, batch] is the moving operand, so each matmul streams 512 batch
columns and yields a time-major [M, batch] output plane. Both planes put
the chunk-end state [p_end, v_end] in output columns 0,1 (the vel
plane's are unused dups) and the real outputs in columns 2..M-1. The
chunk boundary is a single gpsimd (SWDGE) DMA straight out of the pos
plane's PSUM rows 0:2 into the next stack's fp16 carry rows — SWDGE
DMAs cast f32->fp16, so no engine op sits on the serial carry chain and
the Pool queue carries nothing else. The PSUM->SBUF output copies (pos
on ACT, vel on DVE) and the output DMA are off-chain.

Per (chunk, coord): 4 matmuls [128, 512] (pos/vel x 2 batch halves), 2
PSUM->SBUF fp16 copies of [128, 1024], 1 SWDGE carry DMA, 1 output DMA
of [128, 2, 1024] fp16. Inputs stream via sync/scalar HWDGE, two chunks
ahead.

Sharding: pure data parallel over batch across 8 cores (1024 rows/core).
"""

import numpy as np

import concourse.bass as bass
import concourse.tile as tile
from concourse import bacc, mybir
from concourse.bass_utils import run_bass_kernel_spmd

# Problem shape (hardcoded per harness contract)
B = 8192
T = 1024
NCORES = 8
BC = B // NCORES  # 1024 batch rows per core
L = 126  # z rows per full chunk; K = L + 2 <= 128
CH = []
_t0 = 0
while _t0 < T:
    CH.append((_t0, min(L, T - _t0)))
    _t0 += CH[-1][1]
NCH = len(CH)  # 8 full chunks + 16-step tail


# ---------------------------------------------------------------- host math
def _gains(dt, q_pos, q_vel, r_vel):
    """Scalar Kalman gain sequences in float64 (exact vs fp32 reference)."""
    dt = float(dt)
    r_reg = float(np.float32(r_vel) + np.float32(1e-6))
    q_pos = float(q_pos)
    q_vel = float(q_vel)
    a, b, c = 1.0, 0.0, 1.0  # P blocks [[a, b], [b, c]] per coordinate
    kp = np.zeros(T)
    kv = np.zeros(T)
    for t in range(T):
        ap = a + 2 * dt * b + dt * dt * c + q_pos
        bp = b + dt * c
        cp = c + q_vel
        den = cp + r_reg
        kp[t] = bp / den
        kv[t] = cp / den
        a = ap - kp[t] * bp
        b = bp * r_reg / den
        c = cp * r_reg / den
    return kp, kv


def _build_weights(kp, kv, dt):
    """Per-chunk weights W[K, 2, K] (fp16), K = Lc + 2.

    Stack rows: [p_c, v_c, z_0..z_{Lc-1}]. For each plane pl (0=pos,
    1=vel): col 0 = p_end coeffs, col 1 = v_end coeffs, col 2+u = state
    component pl at local step u.
    Full-size chunks are deduped (gains converge -> steady chunks share W).
    """
    dt = float(dt)
    w64 = []
    for t0, Lc in CH:
        K = Lc + 2
        U = np.zeros((K, 2))
        U[0] = (1.0, 0.0)
        U[1] = (0.0, 1.0)
        W = np.zeros((K, 2, K))
        for u in range(Lc):
            t = t0 + u
            M = np.array([[1.0, 0.0], [dt - kp[t], 1.0 - kv[t]]])
            U[: 2 + u] = U[: 2 + u] @ M
            U[2 + u] = (kp[t], kv[t])
            W[: 2 + u + 1, :, 2 + u] = U[: 2 + u + 1]
        W[:, 0, 0:2] = U  # [p_end, v_end] coeff cols (both planes)
        W[:, 1, 0:2] = U
        w64.append(W.astype(np.float16))

    chunk_map = []
    uniq = []
    nfull = sum(1 for _, Lc in CH if Lc == L)
    for c in range(nfull):
        found = None
        for ui, u in enumerate(uniq):
            if np.array_equal(w64[c], w64[u]):
                found = ui
                break
        if found is None:
            uniq.append(c)
            found = len(uniq) - 1
        chunk_map.append(found)

    w_full = np.stack([w64[u] for u in uniq])  # (NU, 128, 2, 128)
    w_last = w64[-1] if CH[-1][1] != L else None  # (Kl, 2, Kl)
    return w_full, w_last, chunk_map


def _split_weights(w_full, w_last, chunk_map):
    """Split into chunk-0 full W, z-part/carry-part for chunks >= 1.

    Chunk 0's carry rows are host-baked into its stack (single matmul);
    later chunks run two accumulating matmuls: Wz over the z-only stack
    and Wc (K=2) over the previous out tile's [p_end, v_end] rows.
    """
    w0 = w_full[chunk_map[0]]  # (128, 2, 128)
    uniq1 = sorted({chunk_map[c] for c in range(1, len(chunk_map))})
    remap = {u: i for i, u in enumerate(uniq1)}
    chunk_map1 = [remap[chunk_map[c]] for c in range(1, len(chunk_map))]
    wz = np.ascontiguousarray(
        np.stack([w_full[u] for u in uniq1]).transpose(1, 0, 2, 3)[2:]
    )  # (126, NU1, 2, 128)
    wc = np.ascontiguousarray(
        np.stack([w_full[u] for u in uniq1]).transpose(1, 0, 2, 3)[0:2]
    )  # (2, NU1, 2, 128)
    if w_last is not None:
        wzl = np.ascontiguousarray(w_last[2:])  # (16, 2, 18)
        wcl = np.ascontiguousarray(w_last[0:2])  # (2, 2, 18)
    else:
        wzl = wcl = None
    return w0, wz, wc, wzl, wcl, chunk_map1


# ---------------------------------------------------------------- bass build
def _build_nc(nu1, chunk_map1, has_last):
    f32 = mybir.dt.float32
    f16 = mybir.dt.float16

    nc = bacc.Bacc(
        "TRN2",
        target_bir_lowering=False,
        debug=False,
        enable_asserts=False,
    )
    KF = L + 2  # 128
    zin_d = nc.dram_tensor("zin", [2, NCH, KF, BC], f16, kind="ExternalInput").ap()
    w0_d = nc.dram_tensor("w0", [KF, 2, KF], f16, kind="ExternalInput").ap()
    wz_d = nc.dram_tensor("wz", [L, nu1, 2, KF], f16, kind="ExternalInput").ap()
    wc_d = nc.dram_tensor("wc", [2, nu1, 2, KF], f16, kind="ExternalInput").ap()
    if has_last:
        _, Ll = CH[-1]
        wzl_d = nc.dram_tensor("wzl", [Ll, 2, Ll + 2], f16, kind="ExternalInput").ap()
        wcl_d = nc.dram_tensor("wcl", [2, 2, Ll + 2], f16, kind="ExternalInput").ap()
    out_d = nc.dram_tensor("out", [2, NCH, L, 2, BC], f16, kind="ExternalOutput").ap()

    with tile.TileContext(nc) as tc:
        with (
            tc.tile_pool(name="wpool", bufs=1) as wpool,
            tc.tile_pool(name="stacks", bufs=1) as spool,
            tc.tile_pool(name="outp", bufs=1) as opool,
            tc.tile_pool(name="mpsum", bufs=4, space="PSUM") as mpsum_pool,
        ):
            # stacks: chunk 0 holds [p_c, v_c | z] (carry host-baked into
            # zin); chunks >= 1 hold z only — their carry contribution is
            # a K=2 matmul reading the previous out tile's rows 0,1.
            # All loads are issued up front: they never wait, and a deep
            # standing queue of input transfers keeps DMA_ENGINES packed.
            stacks = {}

            def make_stack(c, cd):
                Lc = CH[c][1]
                K = Lc + 2 if c == 0 else Lc
                stk = spool.tile(
                    [K, BC], f16, tag=f"stk_{c}_{cd}", name=f"stk_{c}_{cd}"
                )
                eng = nc.scalar if cd else nc.sync
                if c == 0:
                    eng.dma_start(stk[0:K, :], zin_d[cd, c, 0:K, :])
                else:
                    eng.dma_start(stk[0:K, :], zin_d[cd, c, 2 : 2 + K, :])
                stacks[(c, cd)] = stk

            # warm the ACT activation table off the critical path
            warm = wpool.tile([1, 8], f16)
            nc.vector.memset(warm[:], 0.0)
            nc.scalar.mul(warm[:], warm[:], 1.0)

            for cd in range(2):
                make_stack(0, cd)

            w0_t = wpool.tile([KF, 2, KF], f16)
            nc.sync.dma_start(w0_t[:], w0_d)
            wz_t = wpool.tile([L, nu1, 2, KF], f16)
            nc.scalar.dma_start(wz_t[:], wz_d)
            wc_t = wpool.tile([2, nu1, 2, KF], f16)
            nc.sync.dma_start(wc_t[:], wc_d)
            if has_last:
                _, Ll = CH[-1]
                wzl_t = wpool.tile([Ll, 2, Ll + 2], f16)
                nc.scalar.dma_start(wzl_t[:], wzl_d)
                wcl_t = wpool.tile([2, 2, Ll + 2], f16)
                nc.sync.dma_start(wcl_t[:], wcl_d)

            for cd in range(2):
                make_stack(1, cd)

            outs = {}
            for c in range(NCH):
                K = CH[c][1] + 2
                for cd in range(2):
                    outs[(c, cd)] = opool.tile(
                        [K, 2, BC], f16, tag=f"out_{c}_{cd}", name=f"out_{c}_{cd}"
                    )

            def wzslice(c, pl):
                if CH[c][1] == L:
                    return wz_t[:, chunk_map1[c - 1], pl, :]
                return wzl_t[:, pl, :]

            def wcslice(c, pl):
                if CH[c][1] == L:
                    return wc_t[:, chunk_map1[c - 1], pl, :]
                return wcl_t[:, pl, :]

            for c in range(NCH):
                t0, Lc = CH[c]
                M = Lc + 2
                for cd in range(2):
                    stk = stacks[(c, cd)]
                    out_t = outs[(c, cd)]
                    pss = []
                    for pl in range(2):
                        ps = mpsum_pool.tile([M, BC], f32, tag="ps")
                        for h in range(2):
                            hsl = slice(h * 512, (h + 1) * 512)
                            if c == 0:
                                nc.tensor.matmul(
                                    ps[:, hsl], w0_t[:, pl, :], stk[:, hsl],
                                    start=True, stop=True,
                                )
                            else:
                                nc.tensor.matmul(
                                    ps[:, hsl], wzslice(c, pl), stk[:, hsl],
                                    start=True, stop=False,
                                )
                                # carry: prev out tile rows 0,1 of pos plane
                                nc.tensor.matmul(
                                    ps[:, hsl], wcslice(c, pl),
                                    outs[(c - 1, cd)][0:2, 0, hsl],
                                    start=False, stop=True,
                                )
                        pss.append(ps)
                    # pos copy on ACT (feeds next chunk's carry matmuls),
                    # vel on DVE
                    nc.scalar.mul(out_t[:, 0, :], pss[0][:], 1.0)
                    nc.vector.tensor_scalar_mul(out_t[:, 1, :], pss[1][:], 1.0)
                # prefetch inputs two chunks ahead
                if c + 2 < NCH:
                    for cd in range(2):
                        make_stack(c + 2, cd)
                # out DMAs after both coords' copies so an out DMA's wait
                # (held on the issuing SEQ) never blocks the next pos copy
                for cd in range(2):
                    Lc = CH[c][1]
                    eng = nc.scalar if cd else nc.sync
                    eng.dma_start(
                        out_d[cd, c, 0:Lc, :, :], outs[(c, cd)][2 : 2 + Lc, :, :]
                    )
    nc.compile()
    return nc


# ---------------------------------------------------------------- entry
def _prepare(pred_vel, dt, p0, q_pos, q_vel, r_vel):
    kp, kv = _gains(dt, q_pos, q_vel, r_vel)
    w_full, w_last, chunk_map = _build_weights(kp, kv, dt)
    w0, wz, wc, wzl, wcl, chunk_map1 = _split_weights(w_full, w_last, chunk_map)
    nu1 = wz.shape[1]

    pred_vel = np.asarray(pred_vel, dtype=np.float32)
    p0 = np.asarray(p0, dtype=np.float32)
    in_maps = []
    for i in range(NCORES):
        pv = pred_vel[i * BC : (i + 1) * BC]  # (BC, T, 2)
        zt = np.ascontiguousarray(pv.transpose(2, 1, 0)).astype(np.float16)
        zin = np.zeros((2, NCH, L + 2, BC), dtype=np.float16)
        for c, (t0, Lc) in enumerate(CH):
            zin[:, c, 2 : 2 + Lc, :] = zt[:, t0 : t0 + Lc, :]
        # chunk 0 carry rows: p = p0, v = 0
        zin[:, 0, 0, :] = p0[i * BC : (i + 1) * BC].T.astype(np.float16)
        m = {"zin": zin, "w0": w0, "wz": wz, "wc": wc}
        if wzl is not None:
            m["wzl"] = wzl
            m["wcl"] = wcl
        in_maps.append(m)
    return nu1, chunk_map1, wzl is not None, in_maps


def run(pred_vel, dt, p0, q_pos, q_vel, r_vel, trace=False, **spmd_kwargs):
    nu1, chunk_map1, has_last, in_maps = _prepare(
        pred_vel, dt, p0, q_pos, q_vel, r_vel
    )
    nc = _build_nc(nu1, chunk_map1, has_last)
    res = run_bass_kernel_spmd(
        nc, in_maps, core_ids=list(range(NCORES)), trace=trace, **spmd_kwargs
    )
    pos = np.empty((B, T, 2), dtype=np.float32)
    vel = np.empty((B, T, 2), dtype=np.float32)
    for i in range(NCORES):
        o = res.results[i]["out"]  # (2, NCH, 126, 2, BC) fp16
        sl = slice(i * BC, (i + 1) * BC)
        for c, (t0, Lc) in enumerate(CH):
            oc = o[:, c, 0:Lc, :, :].astype(np.float32)  # (2,Lc,2,BC)
            for cd in range(2):
                pos[sl, t0 : t0 + Lc, cd] = oc[cd, :, 0, :].T
                vel[sl, t0 : t0 + Lc, cd] = oc[cd, :, 1, :].T
    return (pos, vel), res


def kernel(pred_vel, dt, p0, q_pos, q_vel, r_vel):
    (pos, vel), _ = run(pred_vel, dt, p0, q_pos, q_vel, r_vel, trace=False)
    return pos, vel


# revision 3
# speedup vs baseline: 1.0888x; 1.0378x over previous
"""Trainium2 Bass kernel for nn_DifferentiableKalmanFilter (v2).

Strategy
--------
The 4x4 covariance recursion is batch-independent and, by x/y symmetry,
collapses to two scalar gain sequences k_p(t), k_v(t) computed on the host.
Per batch row the filter is a 2-state linear recurrence per coordinate:

    s_t = s_{t-1} @ M_t + z_t * g_t,   s = [p, v],
    M_t = [[1, 0], [dt - k_p(t), 1 - k_v(t)]],  g_t = [k_p(t), k_v(t)]

Unrolled over a chunk of L=126 timesteps this is a matmul with
host-precomputed weights. The tolerance (2e-2) allows a single fp16
product (no hi/lo splitting, no scaling): z, W, the carried state and the
output are all plain fp16; PSUM accumulates in f32.

Layout: weights are the stationary operand [K=2+L, M=2+L], the z/carry
stack [K, batch] is the moving operand, so each matmul streams 512 batch
columns and yields a time-major [M, batch] output plane. Both planes put
the chunk-end state [p_end, v_end] in output columns 0,1 (the vel
plane's are unused dups) and the real outputs in columns 2..M-1. The
chunk boundary is a single gpsimd (SWDGE) DMA straight out of the pos
plane's PSUM rows 0:2 into the next stack's fp16 carry rows — SWDGE
DMAs cast f32->fp16, so no engine op sits on the serial carry chain and
the Pool queue carries nothing else. The PSUM->SBUF output copies (pos
on ACT, vel on DVE) and the output DMA are off-chain.

Per (chunk, coord): 4 matmuls [128, 512] (pos/vel x 2 batch halves), 2
PSUM->SBUF fp16 copies of [128, 1024], 1 SWDGE carry DMA, 1 output DMA
of [128, 2, 1024] fp16. Inputs stream via sync/scalar HWDGE, two chunks
ahead.

Sharding: pure data parallel over batch across 8 cores (1024 rows/core).
"""

import numpy as np

import concourse.bass as bass
import concourse.tile as tile
from concourse import bacc, mybir
from concourse.bass_utils import run_bass_kernel_spmd

# Problem shape (hardcoded per harness contract)
B = 8192
T = 1024
NCORES = 8
BC = B // NCORES  # 1024 batch rows per core
L = 126  # z rows per full chunk; K = L + 2 <= 128
CH = []
_t0 = 0
while _t0 < T:
    CH.append((_t0, min(L, T - _t0)))
    _t0 += CH[-1][1]
NCH = len(CH)  # 8 full chunks + 16-step tail


# ---------------------------------------------------------------- host math
def _gains(dt, q_pos, q_vel, r_vel):
    """Scalar Kalman gain sequences in float64 (exact vs fp32 reference)."""
    dt = float(dt)
    r_reg = float(np.float32(r_vel) + np.float32(1e-6))
    q_pos = float(q_pos)
    q_vel = float(q_vel)
    a, b, c = 1.0, 0.0, 1.0  # P blocks [[a, b], [b, c]] per coordinate
    kp = np.zeros(T)
    kv = np.zeros(T)
    for t in range(T):
        ap = a + 2 * dt * b + dt * dt * c + q_pos
        bp = b + dt * c
        cp = c + q_vel
        den = cp + r_reg
        kp[t] = bp / den
        kv[t] = cp / den
        a = ap - kp[t] * bp
        b = bp * r_reg / den
        c = cp * r_reg / den
    return kp, kv


def _build_weights(kp, kv, dt):
    """Per-chunk weights W[K, 2, K] (fp16), K = Lc + 2.

    Stack rows: [p_c, v_c, z_0..z_{Lc-1}]. For each plane pl (0=pos,
    1=vel): col 0 = p_end coeffs, col 1 = v_end coeffs, col 2+u = state
    component pl at local step u.
    Full-size chunks are deduped (gains converge -> steady chunks share W).
    """
    dt = float(dt)
    w64 = []
    for t0, Lc in CH:
        K = Lc + 2
        U = np.zeros((K, 2))
        U[0] = (1.0, 0.0)
        U[1] = (0.0, 1.0)
        W = np.zeros((K, 2, K))
        for u in range(Lc):
            t = t0 + u
            M = np.array([[1.0, 0.0], [dt - kp[t], 1.0 - kv[t]]])
            U[: 2 + u] = U[: 2 + u] @ M
            U[2 + u] = (kp[t], kv[t])
            W[: 2 + u + 1, :, 2 + u] = U[: 2 + u + 1]
        W[:, 0, 0:2] = U  # [p_end, v_end] coeff cols (both planes)
        W[:, 1, 0:2] = U
        w64.append(W.astype(np.float16))

    chunk_map = []
    uniq = []
    nfull = sum(1 for _, Lc in CH if Lc == L)
    for c in range(nfull):
        found = None
        for ui, u in enumerate(uniq):
            if np.array_equal(w64[c], w64[u]):
                found = ui
                break
        if found is None:
            uniq.append(c)
            found = len(uniq) - 1
        chunk_map.append(found)

    w_full = np.stack([w64[u] for u in uniq])  # (NU, 128, 2, 128)
    w_last = w64[-1] if CH[-1][1] != L else None  # (Kl, 2, Kl)
    return w_full, w_last, chunk_map


def _split_weights(w_full, w_last, chunk_map):
    """Split into chunk-0 full W, z-part/carry-part for chunks >= 1.

    Chunk 0's carry rows are host-baked into its stack (single matmul);
    later chunks run two accumulating matmuls: Wz over the z-only stack
    and Wc (K=2) over the previous out tile's [p_end, v_end] rows.
    """
    w0 = w_full[chunk_map[0]]  # (128, 2, 128)
    uniq1 = sorted({chunk_map[c] for c in range(1, len(chunk_map))})
    remap = {u: i for i, u in enumerate(uniq1)}
    chunk_map1 = [remap[chunk_map[c]] for c in range(1, len(chunk_map))]
    wz = np.ascontiguousarray(
        np.stack([w_full[u] for u in uniq1]).transpose(1, 0, 2, 3)[2:]
    )  # (126, NU1, 2, 128)
    wc = np.ascontiguousarray(
        np.stack([w_full[u] for u in uniq1]).transpose(1, 0, 2, 3)[0:2]
    )  # (2, NU1, 2, 128)
    if w_last is not None:
        wzl = np.ascontiguousarray(w_last[2:])  # (16, 2, 18)
        wcl = np.ascontiguousarray(w_last[0:2])  # (2, 2, 18)
    else:
        wzl = wcl = None
    return w0, wz, wc, wzl, wcl, chunk_map1


# ---------------------------------------------------------------- bass build
def _build_nc(nu1, chunk_map1, has_last, vsc):
    f32 = mybir.dt.float32
    f16 = mybir.dt.float16

    nc = bacc.Bacc(
        "TRN2",
        target_bir_lowering=False,
        debug=False,
        enable_asserts=False,
    )
    KF = L + 2  # 128
    zin_d = nc.dram_tensor("zin", [2, NCH, KF, BC], f16, kind="ExternalInput").ap()
    w0_d = nc.dram_tensor("w0", [KF, 2, KF], f16, kind="ExternalInput").ap()
    wz_d = nc.dram_tensor("wz", [L, nu1, 2, KF], f16, kind="ExternalInput").ap()
    wc_d = nc.dram_tensor("wc", [2, nu1, 2, KF], f16, kind="ExternalInput").ap()
    if has_last:
        _, Ll = CH[-1]
        wzl_d = nc.dram_tensor("wzl", [Ll, 2, Ll + 2], f16, kind="ExternalInput").ap()
        wcl_d = nc.dram_tensor("wcl", [2, 2, Ll + 2], f16, kind="ExternalInput").ap()
    outp_d = nc.dram_tensor("outp", [2, NCH, L, BC], f16, kind="ExternalOutput").ap()
    outv_d = nc.dram_tensor(
        "outv", [2, NCH, L, BC], mybir.dt.int8, kind="ExternalOutput"
    ).ap()

    with tile.TileContext(nc) as tc:
        with (
            tc.tile_pool(name="wpool", bufs=1) as wpool,
            tc.tile_pool(name="stacks", bufs=1) as spool,
            tc.tile_pool(name="outp", bufs=1) as opool,
            tc.tile_pool(name="mpsum", bufs=4, space="PSUM") as mpsum_pool,
        ):
            # stacks: chunk 0 holds [p_c, v_c | z] (carry host-baked into
            # zin); chunks >= 1 hold z only — their carry contribution is
            # a K=2 matmul reading the previous out tile's rows 0,1.
            # All loads are issued up front: they never wait, and a deep
            # standing queue of input transfers keeps DMA_ENGINES packed.
            stacks = {}

            def make_stack(c, cd):
                Lc = CH[c][1]
                K = Lc + 2 if c == 0 else Lc
                stk = spool.tile(
                    [K, BC], f16, tag=f"stk_{c}_{cd}", name=f"stk_{c}_{cd}"
                )
                if c == 0:
                    eng = nc.scalar if cd else nc.sync
                    eng.dma_start(stk[0:K, :], zin_d[cd, c, 0:K, :])
                else:
                    nc.gpsimd.dma_start(stk[0:K, :], zin_d[cd, c, 2 : 2 + K, :])
                stacks[(c, cd)] = stk

            # warm the ACT activation table off the critical path
            warm = wpool.tile([1, 8], f16)
            nc.vector.memset(warm[:], 0.0)
            nc.scalar.mul(warm[:], warm[:], 1.0)

            # W0 first: its transfer is tiny, so the first matmul's
            # longest pole (the chunk-0 stack) lands last
            w0_t = wpool.tile([KF, 2, KF], f16)
            nc.sync.dma_start(w0_t[:], w0_d)

            for cd in range(2):
                make_stack(0, cd)

            wz_t = wpool.tile([L, nu1, 2, KF], f16)
            nc.scalar.dma_start(wz_t[:], wz_d)
            wc_t = wpool.tile([2, nu1, 2, KF], f16)
            nc.sync.dma_start(wc_t[:], wc_d)
            if has_last:
                _, Ll = CH[-1]
                wzl_t = wpool.tile([Ll, 2, Ll + 2], f16)
                nc.scalar.dma_start(wzl_t[:], wzl_d)
                wcl_t = wpool.tile([2, 2, Ll + 2], f16)
                nc.sync.dma_start(wcl_t[:], wcl_d)

            for c in range(1, min(3, NCH)):
                for cd in range(2):
                    make_stack(c, cd)

            outps = {}
            outvs = {}
            for c in range(NCH):
                K = CH[c][1] + 2
                for cd in range(2):
                    outps[(c, cd)] = opool.tile(
                        [K, BC], f16, tag=f"outp_{c}_{cd}", name=f"outp_{c}_{cd}"
                    )
                    outvs[(c, cd)] = opool.tile(
                        [K, BC], mybir.dt.int8,
                        tag=f"outv_{c}_{cd}", name=f"outv_{c}_{cd}",
                    )

            def wzslice(c, pl):
                if CH[c][1] == L:
                    return wz_t[:, chunk_map1[c - 1], pl, :]
                return wzl_t[:, pl, :]

            def wcslice(c, pl):
                if CH[c][1] == L:
                    return wc_t[:, chunk_map1[c - 1], pl, :]
                return wcl_t[:, pl, :]

            for c in range(NCH):
                t0, Lc = CH[c]
                M = Lc + 2
                for cd in range(2):
                    stk = stacks[(c, cd)]
                    pss = []
                    for pl in range(2):
                        ps = mpsum_pool.tile([M, BC], f32, tag="ps")
                        for h in range(2):
                            hsl = slice(h * 512, (h + 1) * 512)
                            if c == 0:
                                nc.tensor.matmul(
                                    ps[:, hsl], w0_t[:, pl, :], stk[:, hsl],
                                    start=True, stop=True,
                                )
                            else:
                                nc.tensor.matmul(
                                    ps[:, hsl], wzslice(c, pl), stk[:, hsl],
                                    start=True, stop=False,
                                )
                                # carry: prev out tile rows 0,1 of pos plane
                                nc.tensor.matmul(
                                    ps[:, hsl], wcslice(c, pl),
                                    outps[(c - 1, cd)][0:2, hsl],
                                    start=False, stop=True,
                                )
                        pss.append(ps)
                    # pos copy on ACT (feeds next chunk's carry matmuls);
                    # vel quantizes to int8 on DVE (|v| <= max|z| makes the
                    # host-computed scale sound, so no saturation)
                    nc.scalar.mul(outps[(c, cd)][:], pss[0][:], 1.0)
                    nc.vector.tensor_scalar_mul(outvs[(c, cd)][:], pss[1][:], vsc)
                # prefetch inputs three chunks ahead
                if c + 3 < NCH:
                    for cd in range(2):
                        make_stack(c + 3, cd)
                # out DMAs are emitted one chunk late: their copies are
                # long done by then, so the issuing SEQ never stalls
                # holding the queue while a copy drains
                for cp in ([c - 1] if c else []) + ([c] if c == NCH - 1 else []):
                    for cd in range(2):
                        Lc = CH[cp][1]
                        eng = nc.scalar if cd else nc.sync
                        oth = nc.sync if cd else nc.scalar
                        eng.dma_start(
                            outp_d[cd, cp, 0:Lc, :], outps[(cp, cd)][2 : 2 + Lc, :]
                        )
                        oth.dma_start(
                            outv_d[cd, cp, 0:Lc, :], outvs[(cp, cd)][2 : 2 + Lc, :]
                        )
    nc.compile()
    return nc


# ---------------------------------------------------------------- entry
def _prepare(pred_vel, dt, p0, q_pos, q_vel, r_vel):
    kp, kv = _gains(dt, q_pos, q_vel, r_vel)
    w_full, w_last, chunk_map = _build_weights(kp, kv, dt)
    w0, wz, wc, wzl, wcl, chunk_map1 = _split_weights(w_full, w_last, chunk_map)
    nu1 = wz.shape[1]

    pred_vel = np.asarray(pred_vel, dtype=np.float32)
    p0 = np.asarray(p0, dtype=np.float32)
    vsc = 126.0 / max(float(np.abs(pred_vel).max()), 1e-30)
    in_maps = []
    for i in range(NCORES):
        pv = pred_vel[i * BC : (i + 1) * BC]  # (BC, T, 2)
        zt = np.ascontiguousarray(pv.transpose(2, 1, 0)).astype(np.float16)
        zin = np.zeros((2, NCH, L + 2, BC), dtype=np.float16)
        for c, (t0, Lc) in enumerate(CH):
            zin[:, c, 2 : 2 + Lc, :] = zt[:, t0 : t0 + Lc, :]
        # chunk 0 carry rows: p = p0, v = 0
        zin[:, 0, 0, :] = p0[i * BC : (i + 1) * BC].T.astype(np.float16)
        m = {"zin": zin, "w0": w0, "wz": wz, "wc": wc}
        if wzl is not None:
            m["wzl"] = wzl
            m["wcl"] = wcl
        in_maps.append(m)
    return nu1, chunk_map1, wzl is not None, vsc, in_maps


def run(pred_vel, dt, p0, q_pos, q_vel, r_vel, trace=False, **spmd_kwargs):
    nu1, chunk_map1, has_last, vsc, in_maps = _prepare(
        pred_vel, dt, p0, q_pos, q_vel, r_vel
    )
    nc = _build_nc(nu1, chunk_map1, has_last, vsc)
    res = run_bass_kernel_spmd(
        nc, in_maps, core_ids=list(range(NCORES)), trace=trace, **spmd_kwargs
    )
    pos = np.empty((B, T, 2), dtype=np.float32)
    vel = np.empty((B, T, 2), dtype=np.float32)
    inv = 1.0 / vsc
    for i in range(NCORES):
        op = res.results[i]["outp"]  # (2, NCH, 126, BC) fp16
        ov = res.results[i]["outv"]  # (2, NCH, 126, BC) int8
        sl = slice(i * BC, (i + 1) * BC)
        for c, (t0, Lc) in enumerate(CH):
            for cd in range(2):
                pos[sl, t0 : t0 + Lc, cd] = (
                    op[cd, c, 0:Lc, :].astype(np.float32).T
                )
                vel[sl, t0 : t0 + Lc, cd] = (
                    ov[cd, c, 0:Lc, :].astype(np.float32).T * inv
                )
    return (pos, vel), res


def kernel(pred_vel, dt, p0, q_pos, q_vel, r_vel):
    (pos, vel), _ = run(pred_vel, dt, p0, q_pos, q_vel, r_vel, trace=False)
    return pos, vel
